# revision 58
# baseline (speedup 1.0000x reference)
"""Causal self-attention (S=2048, D=1024, H=16) on 8 Trainium2 NeuronCores.

Sharding: tensor-parallel over heads; core c owns heads 2c, 2c+1.

v3 pipeline (fp8 DoubleRow q/k path, bf16 value path):
  - q,k projections run as fp8e4 DoubleRow matmuls (256-row contraction per
    instruction, 0.5 PE cycles/row): hs and W_qk stream as fp8 (W scaled x32
    so its mass sits in e4m3's normal range). psum -> fp8 bias-cast, then two
    sbuf->sbuf DMAs repack [128, 512] into the [32-partition, 2, S]-per-head
    DoubleRow layout.
  - QK scores are fp8 DoubleRow too: per (head, kb) one [32,2,128]x[32,2,512]
    matmul. Score psum tiles are keyed by kb with the two heads side by side
    in the free dim, so the h0/h1 matmuls (PE row-tiles (0,0)/(32,0)) sit
    adjacent in the PE stream and dual-issue: 216ns per 2-head 128x512 score
    block, measured. Logits come out x1024; the exp scale absorbs it. The
    softmax compresses the fp8 quantization to ~0.5% output error because
    logits are tiny (std ~0.07).
  - Causal masking via one PE matmul per diagonal 128x128 sub-block: lhsT is
    a -1e7 strict-upper-triangle, rhs a paired identity, accumulated onto
    both heads' scores before exp (exp underflows masked entries to 0). The
    exp also column-slices each kb to the causally-live region.
  - v projection, PV, and the output projection stay bf16 (fp8 would put its
    full ~4% quantization on the value path; the rel-err gate is 2e-2).
    W_proj is bf16 (fp32r matmuls measured 2x slower than bf16 on hw).
  - Denominators for both heads collected at partitions 0/32 of one tile so
    each qc's Ln+Exp reciprocal is 2 activations; K=1 f32r matmuls broadcast
    the reciprocal rows; DVE multiplies produce bf16 u2n for the projection.
  - PE p-state management: junk matmuls on uninitialized sbuf bridge the
    DMA-bound startup and the tail norm chains (every PE idle drops the
    clock to ~1.2GHz for the next ~3us of work, measured).
  - Input DMAs fan out over the 3 hwdge queues (~100GB/s each, FIFO): sync
    carries hs8 chunk 0/1 + the latency-critical q8/k8 relayouts, scalar the
    chunk-0 weights + half of hs chunk 0, gpsimd small consts + the rest.
    The tail finalizes in 128-col steps: norm chains first, then the last
    four projections, psum copies split scalar/vector, one DMA per
    128-row (or half) slice.
"""

import math
from collections import deque
from contextlib import ExitStack

import numpy as np

import concourse.bacc as bacc
import concourse.mybir as mybir
import concourse.tile as tile
from concourse.bass_utils import run_bass_kernel_spmd

S, D, H = 2048, 1024, 16
HS = D // H  # 64 head size
P = 128
NCORES = 8
HPC = H // NCORES  # 2 heads per core
CD = HPC * HS  # 128 per-core head dims
KO = D // P  # 8 contraction tiles
NQC = S // 512  # 4 query chunks
NSC = S // P  # 16 sequence chunks of 128
SCALE = 1.0 / math.sqrt(S)
WSCALE = 32.0  # fp8 scale for W_qk (and thus q,k); exp scale divides it out

F32 = mybir.dt.float32
F32R = mybir.dt.float32r
BF16 = mybir.dt.bfloat16
FP8 = mybir.dt.float8e4
DR = mybir.MatmulPerfMode.DoubleRow

import ml_dtypes

NP_BF16 = ml_dtypes.bfloat16
NP_FP8 = ml_dtypes.float8_e4m3


def _build():
    nc = bacc.Bacc(
        "TRN2", target_bir_lowering=False, debug=False, num_devices=NCORES
    )

    hsTd = nc.dram_tensor("hsTd", [P, NQC, KO, 512], BF16, kind="ExternalInput")
    hs8d = nc.dram_tensor("hs8d", [P, NQC, KO, 512], FP8, kind="ExternalInput")
    wv_d = nc.dram_tensor("wv_d", [P, KO, P], BF16, kind="ExternalInput")
    wqk8d = nc.dram_tensor("wqk8d", [P, KO, 2 * P], FP8, kind="ExternalInput")
    b_qkv = nc.dram_tensor("b_qkv", [P, 3], F32, kind="ExternalInput")
    w_p = nc.dram_tensor("w_p", [CD, D], BF16, kind="ExternalInput")
    masku = nc.dram_tensor("masku", [P, P], BF16, kind="ExternalInput")
    iden2 = nc.dram_tensor("iden2", [P, 2, P], BF16, kind="ExternalInput")
    iden_b = nc.dram_tensor("iden_b", [P, P], BF16, kind="ExternalInput")
    vones = nc.dram_tensor("vones", [P, NSC, HPC], BF16, kind="ExternalInput")
    ones_r = nc.dram_tensor("ones_r", [33, P], F32R, kind="ExternalInput")
    out = nc.dram_tensor("out", [S, D], BF16, kind="ExternalOutput")

    with (
        tile.TileContext(nc) as tc,
        ExitStack() as ctx,
        nc.allow_low_precision(reason="fp8/bf16 matmul pipeline"),
    ):
        const = ctx.enter_context(tc.tile_pool(name="const", bufs=1))
        work = ctx.enter_context(tc.tile_pool(name="work", bufs=2))
        pp = ctx.enter_context(tc.tile_pool(name="pp", bufs=1, space="PSUM"))

        def psA(name):  # shared 2-bank psum ring
            return pp.tile([P, 2, 512], F32, tag="A", bufs=3, name=name)

        # (PE warm-up is emitted right after the vT_sb tile exists below)

        # ---- loads, fanned over 3 queues (~100 GB/s each, FIFO per queue).
        # scalar: q/k weights + v weights + bias + hs0 high half;
        # sync: hs8 chunk 0 then (in emission order) the relayout DMAs and
        # hs8 chunk 1 so the fp8 attention path is never behind bulk;
        # gpsimd: small consts, hs0 low half, then the remaining bulk.
        wqk8_sb = const.tile([P, KO, 2 * P], FP8, tag="wqk8", name="wqk8_sb")
        nc.scalar.dma_start(out=wqk8_sb, in_=wqk8d.ap())
        hs_n = [
            const.tile([P, KO, 512], BF16, tag=f"hs{n}", name=f"hs{n}")
            for n in range(NQC)
        ]
        nc.scalar.dma_start(out=hs_n[0][:, 0:4, :], in_=hsTd.ap()[:, 0, 0:4, :])

        hs8_n = [
            const.tile([P, KO, 512], FP8, tag=f"hs8_{n}", name=f"hs8_{n}")
            for n in range(NQC)
        ]
        nc.sync.dma_start(out=hs8_n[0], in_=hs8d.ap()[:, 0, :, :])

        # gpsimd's software DGE costs ~1us fixed PER DESCRIPTOR, so this
        # queue is ordered strictly by need-time: v weights and the mask
        # tensors come before the late-needed consts and all bulk
        bqkv_sb = const.tile([P, 3], F32, tag="bqkv", name="bqkv_sb")
        nc.gpsimd.dma_start(out=bqkv_sb, in_=b_qkv.ap())
        wv_sb = const.tile([P, KO, P], BF16, tag="wv", name="wv_sb")
        nc.gpsimd.dma_start(out=wv_sb, in_=wv_d.ap())
        masku_sb = const.tile([P, P], BF16, tag="masku", name="masku_sb")
        nc.gpsimd.dma_start(out=masku_sb, in_=masku.ap())
        iden2_sb = const.tile([P, 2, P], BF16, tag="iden2", name="iden2_sb")
        nc.gpsimd.dma_start(out=iden2_sb, in_=iden2.ap())
        v2_sb = const.tile([P, NSC, HPC, HS + 1], BF16, tag="v2", name="v2_sb")
        nc.gpsimd.dma_start(out=v2_sb[:, :, :, HS], in_=vones.ap())
        nc.gpsimd.dma_start(out=hs_n[0][:, 4:8, :], in_=hsTd.ap()[:, 0, 4:8, :])
        identb = const.tile([P, P], BF16, tag="identb", name="identb")
        nc.gpsimd.dma_start(out=identb, in_=iden_b.ap())
        onesr_sb = const.tile([33, P], F32R, tag="onesr", name="onesr_sb")
        nc.gpsimd.dma_start(out=onesr_sb, in_=ones_r.ap())
        nc.gpsimd.dma_start(out=hs_n[1], in_=hsTd.ap()[:, 1, :, :])
        nc.gpsimd.dma_start(out=hs8_n[2], in_=hs8d.ap()[:, 2, :, :])
        nc.gpsimd.dma_start(out=hs_n[2], in_=hsTd.ap()[:, 2, :, :])
        nc.gpsimd.dma_start(out=hs8_n[3], in_=hs8d.ap()[:, 3, :, :])
        nc.gpsimd.dma_start(out=hs_n[3], in_=hsTd.ap()[:, 3, :, :])

        wp_sb = const.tile([P, D], BF16, tag="wp", name="wp_sb")
        q8_sb = const.tile([CD // 2, 2, S], FP8, tag="q8", name="q8_sb")
        k8_sb = const.tile([CD // 2, 2, S], FP8, tag="k8", name="k8_sb")
        vT_sb = const.tile([P, S], BF16, tag="vT", name="vT_sb")

        # ---- PE warm-up: reads vT_sb junk (written much later by phase 1,
        # so no DMA dependency) to ramp the PE p-state while loads stream
        def junk(n, cols=512):
            ps_w = psA("ps_w")
            for rep in range(n):
                nc.tensor.matmul(
                    ps_w[:, 0, 0:cols],
                    lhsT=vT_sb[:, 0:P],
                    rhs=vT_sb[:, 0:cols],
                    start=True,
                    stop=True,
                )

        junk(16)
        u2_sb = [
            const.tile([P, 512], F32R, tag=f"u2_{qc}", name=f"u2_{qc}")
            for qc in range(NQC)
        ]
        u2n_sb = [
            const.tile([P, 512], BF16, tag=f"u2n_{qc}", name=f"u2n_{qc}")
            for qc in range(NQC)
        ]
        den2_sb = [
            const.tile([33, 512], F32, tag=f"den_{qc}", name=f"den_{qc}")
            for qc in range(NQC)
        ]

        # preload the ln+exp activation table set once (no thrash later)
        from concourse.hw_specs import get_activation_tables

        _tables = list(get_activation_tables(nc.m.arch).keys())
        nc.scalar.add_instruction(
            mybir.InstLoadActFuncSet(
                name=nc.get_next_instruction_name(),
                ins=[],
                outs=[],
                act_func_set_id=_tables.index("natural_log_exp_and_others"),
            )
        )

        # ---- phase 1: q,k fp8 DoubleRow projections + relayout DMAs; v bf16
        # projection into contraction layout + PE transposes into key layout
        def emit_p1_qk(n, m):
            ps_m = psA("ps_qk8")[:, 0, :]
            for o in range(KO // 2):
                nc.tensor.matmul(
                    ps_m,
                    lhsT=wqk8_sb[:, 2 * o : 2 * o + 2, m * P : (m + 1) * P],
                    rhs=hs8_n[n][:, 2 * o : 2 * o + 2, :],
                    start=(o == 0),
                    stop=(o == KO // 2 - 1),
                    perf_mode=DR,
                )
            raw = work.tile([P, 512], FP8, tag=f"qkr{m}", bufs=2, name="qkr")
            nc.vector.tensor_scalar_add(
                out=raw, in0=ps_m, scalar1=bqkv_sb[:, m : m + 1]
            )
            dst = q8_sb if m == 0 else k8_sb
            cols = slice(n * 512, (n + 1) * 512)
            nc.sync.dma_start(out=dst[:, 0, cols], in_=raw[0:64, :])
            nc.sync.dma_start(out=dst[:, 1, cols], in_=raw[64:128, :])

        def emit_p1_v(n):
            ps_m = psA("ps_v")[:, 0, :]
            for o in range(KO):
                nc.tensor.matmul(
                    ps_m,
                    lhsT=wv_sb[:, o, :],
                    rhs=hs_n[n][:, o, :],
                    start=(o == 0),
                    stop=(o == KO - 1),
                )
            nc.vector.tensor_scalar_add(
                out=vT_sb[:, n * 512 : (n + 1) * 512],
                in0=ps_m,
                scalar1=bqkv_sb[:, 2:3],
            )

        def emit_p1_t(sc):
            ps_t = pp.tile([P, P], BF16, tag="A", bufs=3, name="ps_t")
            nc.tensor.transpose(ps_t, vT_sb[:, sc * P : (sc + 1) * P], identb)
            nc.vector.tensor_copy(
                out=v2_sb[:, sc, :, 0:HS],
                in_=ps_t.rearrange("p (a b) -> p a b", a=HPC),
            )

        emit_p1_qk(0, 0)
        emit_p1_qk(0, 1)
        # hs8 chunk 1 rides the sync queue right behind chunk 0's relayouts
        nc.sync.dma_start(out=hs8_n[1], in_=hs8d.ap()[:, 1, :, :])
        emit_p1_v(0)
        for sc in range(4):
            emit_p1_t(sc)

        # ---- norm: den rows for h0/h1 sit at partitions 0/32 of den2, so one
        # Ln + one Exp(-x) covers both heads; K=1 matmul broadcasts the
        # reciprocal rows, one DVE multiply per head. Norm for chunk qc is
        # emitted one group into qc+1 so the PE queue never waits.
        def emit_norm(qc, c0=0, c1=512):
            w = c1 - c0
            lg = work.tile([33, 512], F32, tag="lg", bufs=2, name="lg")
            nc.scalar.activation(
                out=lg[:, 0:w],
                in_=den2_sb[qc][:, c0:c1],
                func=mybir.ActivationFunctionType.Ln,
            )
            rr = work.tile([33, 512], F32R, tag="rr", bufs=2, name="rr")
            nc.scalar.activation(
                out=rr[:, 0:w],
                in_=lg[:, 0:w],
                func=mybir.ActivationFunctionType.Exp,
                scale=-1.0,
            )
            rb_ps = psA("ps_rb")
            for h in range(HPC):
                nc.tensor.matmul(
                    rb_ps[0:HS, h, c0:c1],
                    lhsT=onesr_sb[32 * h : 32 * h + 1, 0:HS],
                    rhs=rr[32 * h : 32 * h + 1, 0:w],
                    start=True,
                    stop=True,
                )
            for h in range(HPC):
                nc.vector.tensor_mul(
                    out=u2n_sb[qc][h * HS : (h + 1) * HS, c0:c1],
                    in0=u2_sb[qc][h * HS : (h + 1) * HS, c0:c1],
                    in1=rb_ps[0:HS, h, c0:c1],
                )

        # ---- phase 3: projection chunk + paired bf16 output DMA
        out_t = {}

        def emit_p3(sc):
            qc = sc // 4
            f = sc % 4
            slot = psA("ps_p3")
            for dc in range(2):
                nc.tensor.matmul(
                    slot[:, dc, :],
                    lhsT=u2n_sb[qc][:, f * P : (f + 1) * P],
                    rhs=wp_sb[:, dc * 512 : (dc + 1) * 512],
                    start=True,
                    stop=True,
                )
            pair = sc // 2
            if sc % 2 == 0:
                out_t[pair] = work.tile(
                    [P, 2, 2, 512], BF16, tag="out", bufs=2, name="out_t"
                )
            if sc >= 4 * NQC - 2:
                # last two slices: split the psum copy across scalar and
                # vector and DMA each half as soon as it lands
                nc.scalar.copy(out=out_t[pair][:, sc % 2, 0], in_=slot[:, 0, :])
                nc.vector.tensor_copy(
                    out=out_t[pair][:, sc % 2, 1], in_=slot[:, 1, :]
                )
            elif sc >= 4 * NQC - 4 and sc % 2 == 1:
                nc.scalar.copy(out=out_t[pair][:, sc % 2], in_=slot)
            else:
                nc.vector.tensor_copy(out=out_t[pair][:, sc % 2], in_=slot)
            if sc >= 4 * NQC - 2:
                # the two half-slices ride different queues in parallel
                for dc, eng in ((0, nc.sync), (1, nc.gpsimd)):
                    eng.dma_start(
                        out=out.ap()[
                            sc * P : (sc + 1) * P, dc * 512 : (dc + 1) * 512
                        ],
                        in_=out_t[pair][:, sc % 2, dc],
                    )
            elif sc >= 4 * NQC - 4:
                # tail: one DMA per 128-row slice so the last transfers
                # start as soon as their psum copy lands
                eng = nc.sync if sc % 2 == 0 else nc.gpsimd
                eng.dma_start(
                    out=out.ap()[sc * P : (sc + 1) * P, :],
                    in_=out_t[pair][:, sc % 2].rearrange("p b c -> p (b c)"),
                )
            elif sc % 2 == 1:
                eng = nc.sync if pair % 2 == 0 else nc.gpsimd
                # dram rows r = 128*j + p -> dims (p, j, c) to match src order
                dst = out.ap()[
                    (pair * 2) * P : (pair * 2 + 2) * P, :
                ].rearrange("(j p) c -> p j c", j=2)
                eng.dma_start(
                    out=dst, in_=out_t[pair].rearrange("p a b c -> p a (b c)")
                )

        def copy_out(qc, ps_o, c0, c1, den_scalar=False):
            for h in range(HPC):
                dst = den2_sb[qc][32 * h : 32 * h + 1, c0:c1]
                src = ps_o[h][HS : HS + 1, c0:c1]
                if den_scalar:
                    nc.scalar.copy(out=dst, in_=src)
                else:
                    nc.vector.tensor_copy(out=dst, in_=src)
            for h in range(HPC):
                nc.vector.tensor_copy(
                    out=u2_sb[qc][h * HS : (h + 1) * HS, c0:c1],
                    in_=ps_o[h][0:HS, c0:c1],
                )

        # ---- phase 2: causal attention with deadline-tagged PE fillers
        fillers = deque()

        def drain(k):
            for _ in range(min(k, len(fillers))):
                fillers.popleft()[1]()

        def drain_due(qc):
            rest = deque()
            while fillers:
                d, fn = fillers.popleft()
                if d <= qc:
                    fn()
                else:
                    rest.append((d, fn))
            fillers.extend(rest)

        for qc in range(NQC):
            if qc + 1 < NQC:
                n = qc + 1
                fillers.append((n, lambda n=n: emit_p1_qk(n, 0)))
                fillers.append((n, lambda n=n: emit_p1_qk(n, 1)))
                fillers.append((n, lambda n=n: emit_p1_v(n)))
                for sc in range(4 * n, 4 * n + 4):
                    fillers.append((n, lambda sc=sc: emit_p1_t(sc)))
                if n == 1:
                    fillers.append(
                        (NQC, lambda: nc.sync.dma_start(out=wp_sb, in_=w_p.ap()))
                    )
            drain_due(qc)

            ps_o = [
                pp.tile([P, 512], F32, tag="O", bufs=2, name=f"ps_o{h}")
                for h in range(HPC)
            ]
            nkb = 4 * (qc + 1)
            ngrp = nkb // 2

            def emit_pv(pend, qc=qc, nkb=nkb, ps_o=ps_o):
                pes, kbs = pend
                for h in range(HPC):
                    for j, kb in enumerate(kbs):
                        c0 = 128 * (kb - 4 * qc) if kb >= 4 * qc else 0
                        nc.tensor.matmul(
                            ps_o[h][0 : HS + 1, c0:512],
                            lhsT=v2_sb[:, kb, h, :],
                            rhs=pes[j][:, h, c0:512],
                            start=(kb == 0),
                            stop=(kb == nkb - 1),
                        )

            pending = None
            for g in range(ngrp):
                kbs = [2 * g, 2 * g + 1]
                f0 = 256 if g == ngrp - 1 else 0
                # one psum tile per kb with heads side by side in the free
                # dim: the h0/h1 DoubleRow matmuls then sit adjacent in the
                # PE stream and dual-issue into row tiles (0,0)/(32,0)
                ps_att = [psA(f"ps_att{j}") for j in range(2)]
                for j, kb in enumerate(kbs):
                    for h in range(HPC):
                        nc.tensor.matmul(
                            ps_att[j][:, h, f0:512],
                            lhsT=k8_sb[
                                h * 32 : (h + 1) * 32, :, kb * P : (kb + 1) * P
                            ],
                            rhs=q8_sb[
                                h * 32 : (h + 1) * 32,
                                :,
                                qc * 512 + f0 : (qc + 1) * 512,
                            ],
                            start=True,
                            stop=True,
                            perf_mode=DR,
                        )
                    jj = kb - 4 * qc
                    if jj >= 0:
                        # causal mask: accumulate -1e7 onto the strict upper
                        # triangle of the diagonal 128x128 sub-block (both
                        # heads at once); exp then underflows those to 0
                        c0 = 128 * jj
                        nc.tensor.matmul(
                            ps_att[j][:, :, c0 : c0 + P],
                            lhsT=masku_sb,
                            rhs=iden2_sb,
                            start=False,
                            stop=True,
                            skip_group_check=True,
                        )
                if qc == 0 and g == 0:
                    # startup is DMA-bound: hold the PE p-state through the
                    # load window with junk matmuls (vT_sb is junk here)
                    junk(10)
                if qc == 1 and g == 0:
                    # known wait on the hs8 chunk-1 transfer: keep the clock
                    junk(6)
                if pending is not None:
                    emit_pv(pending)
                if g == 1 and qc >= 1:
                    emit_norm(qc - 1)
                    for sc in range(4 * (qc - 1), 4 * qc):
                        fillers.append((NQC, lambda sc=sc: emit_p3(sc)))
                # lower half of the last chunk is final one group early:
                # overlap its norm + projection with the last group
                if qc == NQC - 1 and g == ngrp - 1:
                    copy_out(qc, ps_o, 0, 256)
                due = sum(1 for dd, _ in fillers if dd <= qc + 1)
                drain(max(1, -(-due // (ngrp - g))))
                pes = []
                for j, kb in enumerate(kbs):
                    # causality: kb contributes nothing to q-cols below the
                    # diagonal block start, and PV slices them away - skip
                    # them in the exp too
                    c0 = max(f0, 128 * (kb - 4 * qc)) if kb >= 4 * qc else f0
                    p_exp = work.tile(
                        [P, 2, 512], BF16, tag=f"pe{j}", bufs=4, name="p_exp"
                    )
                    nc.scalar.activation(
                        out=p_exp[:, :, c0:512],
                        in_=ps_att[j][:, :, c0:512],
                        func=mybir.ActivationFunctionType.Exp,
                        scale=SCALE / (WSCALE * WSCALE),
                    )
                    pes.append(p_exp)
                pending = (pes, kbs)
            if qc == NQC - 1:
                # tail: all three norm chains are emitted before ANY of the
                # remaining projections (so no 1.1us psum copy ever sits in
                # front of LN/EXP on scalar or the muls on vector), with
                # junk matmuls right after PV to hold the PE p-state while
                # the norm chains run.
                drain(len(fillers))
                emit_norm(qc, 0, 256)
                emit_pv(pending)
                copy_out(qc, ps_o, 256, 384, den_scalar=True)
                copy_out(qc, ps_o, 384, 512, den_scalar=True)
                junk(5)
                emit_norm(qc, 256, 384)
                emit_norm(qc, 384, 512)
                for sc in (12, 13, 14, 15):
                    emit_p3(sc)
            else:
                emit_pv(pending)
                copy_out(qc, ps_o, 0, 512)

        drain(len(fillers))

    nc.compile()
    return nc


_NC = None


def _get_nc():
    global _NC
    if _NC is None:
        _NC = _build()
    return _NC


def prepare_inputs(hidden_states, W_attn, b_attn, W_proj, b_proj):
    hs = np.asarray(hidden_states, dtype=np.float32)
    Wa = np.asarray(W_attn, dtype=np.float32)
    ba = np.asarray(b_attn, dtype=np.float32)
    Wp = np.asarray(W_proj, dtype=np.float32)

    hsT = np.ascontiguousarray(hs.T)  # [D, S] f32
    hsTd = np.ascontiguousarray(
        hsT.astype(NP_BF16).reshape(KO, P, NQC, 512).transpose(1, 2, 0, 3)
    )
    hs8d = np.ascontiguousarray(
        hsT.astype(NP_FP8).reshape(KO, P, NQC, 512).transpose(1, 2, 0, 3)
    )

    pcol = np.arange(P)[:, None]
    ccol = np.arange(P)[None, :]
    # masku[p, r] = -1e7 for r > p: via the identity rhs this accumulates
    # -1e7 onto score[r, c] wherever key r > query c (strict upper tri)
    masku = np.where(ccol > pcol, -1.0e7, 0.0).astype(NP_BF16)
    iden2 = np.broadcast_to(np.eye(P, dtype=np.float32), (2, P, P))
    iden2 = np.ascontiguousarray(iden2.transpose(1, 0, 2)).astype(NP_BF16)

    # DoubleRow q/k layout permutation: psum partition order
    # [h0 d0-31 | h1 d0-31 | h0 d32-63 | h1 d32-63]
    perm = np.concatenate(
        [
            np.arange(0, 32),
            np.arange(64, 96),
            np.arange(32, 64),
            np.arange(96, 128),
        ]
    )

    in_maps = []
    for c in range(NCORES):
        q0 = c * CD
        wq = Wa[:, q0 : q0 + CD][:, perm] * WSCALE
        wk = Wa[:, D + q0 : D + q0 + CD][:, perm] * WSCALE
        wqk = np.concatenate([wq, wk], axis=1).astype(NP_FP8)  # [D, 256]
        wqk8d = np.ascontiguousarray(
            wqk.reshape(KO, P, 2 * P).transpose(1, 0, 2)
        )
        wv = Wa[:, 2 * D + q0 : 2 * D + q0 + CD].astype(NP_BF16)
        wv_d = np.ascontiguousarray(wv.reshape(KO, P, P).transpose(1, 0, 2))
        bq = ba[q0 : q0 + CD][perm] * WSCALE
        bk = ba[D + q0 : D + q0 + CD][perm] * WSCALE
        bv = ba[2 * D + q0 : 2 * D + q0 + CD]
        in_maps.append(
            {
                "hsTd": hsTd,
                "hs8d": hs8d,
                "wqk8d": wqk8d,
                "wv_d": wv_d,
                "b_qkv": np.ascontiguousarray(
                    np.stack([bq, bk, bv], axis=1)
                ).astype(np.float32),
                "w_p": np.ascontiguousarray(
                    Wp[q0 : q0 + CD, :].astype(NP_BF16)
                ),
                "masku": masku,
                "iden2": iden2,
                "iden_b": np.eye(P).astype(NP_BF16),
                "ones_r": np.ones((33, P), dtype=np.float32),
                "vones": np.ones((P, NSC, HPC)).astype(NP_BF16),
            }
        )
    return in_maps


def run(inputs, trace=False):
    """Build+run the sharded kernel. Returns (full_output, BassKernelResults)."""
    in_maps = prepare_inputs(**inputs)
    nc = _get_nc()
    res = run_bass_kernel_spmd(
        nc, in_maps, core_ids=list(range(NCORES)), trace=trace
    )
    acc = np.zeros((S, D), dtype=np.float32)
    for c in range(NCORES):
        acc += res.results[c]["out"].astype(np.float32)
    acc += np.asarray(inputs["b_proj"], dtype=np.float32)
    return acc, res


def kernel(**inputs):
    out, _ = run(inputs, trace=False)
    return out
